# revision 13
# baseline (speedup 1.0000x reference)
"""Trainium2 Bass kernel for NeighborAggregation.

Math: for x of shape (b, k=1024, c=512) viewed as a 32x32 grid over k,
the reference computes y[cell t] = s(t) * 8^(t-1024) where s is a sum of 4
circularly-shifted neighbors minus 4x, and returns concat(x, y) on the c axis.
8^(t-1024) underflows to exactly 0.0 in fp32 for t <= 974, so y is nonzero
only for the last 49 k-rows (t = 975..1023), whose neighbor cells all live in
grid rows {0, 28..31} = flat cells [0..31] and [896..1023].

Kernel strategy (pure data parallel, batch 64 -> 8 cores x 8 examples):
  1. One 16 MiB DRAM->DRAM DMA copies x into out[:, :, 0:512].
  2. The 49 nonzero y rows are computed per example as a sparse fp32 matmul
     on the tensor engine: out49 = W1^T @ x[896:1024] + W2^T @ x[0:32], with
     the neighbor coefficients (+1 x4, -4 self) pre-scaled by 8^(t-1024)
     (exact power-of-two scaling) folded into W. Result lands in
     out[:, 975:1024, 512:1024].
  3. The zero region of y is never written: ExternalOutput buffers are
     pre-zeroed by the runner (both native and PJRT paths).
"""

from contextlib import ExitStack

import numpy as np

_B_FULL, _K, _C = 64, 1024, 512
_NCORES = 8
_B = _B_FULL // _NCORES  # examples per core
_N = 32
_HI = 896  # first cell of grid rows 28..31
_NNZ = 49  # cells 975..1023 have nonzero factor
_Y0 = _K - _NNZ  # 975

_cached = {}
_NP1 = 128  # partitions of the high-cell block (cells 896..1023)


def _weights():
    """W1T (128, 49) over cells 896..1023 and W2T (32, 49) over cells 0..31.

    Column o corresponds to output cell k = 975 + o; entries are the neighbor
    coefficients scaled by factor[k] = 8^(k-1024) (exact in fp32).
    """
    t = np.arange(_K)
    factor = (np.float64(2.0) ** (3.0 * (t - _K))).astype(np.float32)
    w1 = np.zeros((_NP1, _NNZ), np.float32)
    w2 = np.zeros((_N, _NNZ), np.float32)
    for o in range(_NNZ):
        k = _Y0 + o
        i, j = divmod(k, _N)
        f = factor[k]
        i1, i2 = (i + 1) % _N, (i - 2) % _N
        jp, jm = (j + 1) % _N, (j - 2) % _N
        for r, q in [(i1, jp), (i1, jm), (i2, jp), (i2, jm)]:
            cell = _N * r + q
            if cell >= _HI:
                w1[cell - _HI, o] += f
            else:
                w2[cell, o] += f
        w1[k - _HI, o] += np.float32(-4.0) * f
    return w1, w2


def _build_nc():
    import concourse.bacc as bacc
    import concourse.mybir as mybir
    import concourse.tile as tile

    nc = bacc.Bacc("TRN2", debug=False, num_devices=_NCORES)
    f32 = mybir.dt.float32
    x_ap = nc.dram_tensor("x", (_B, _K, _C), f32, kind="ExternalInput").ap()
    w_ap = nc.dram_tensor("w", (_NP1, 2 * _NNZ), f32, kind="ExternalInput").ap()
    out_ap = nc.dram_tensor("out", (_B, _K, 2 * _C), f32, kind="ExternalOutput").ap()

    with tile.TileContext(nc) as tc, ExitStack() as ctx:
        pool = ctx.enter_context(tc.tile_pool(name="sbuf", bufs=1))
        psum_pool = ctx.enter_context(tc.tile_pool(name="psum", bufs=4, space="PSUM"))

        # Sync (SP) ring, in FIFO order: the small matmul inputs FIRST so
        # they run at the full 360 GB/s bus rate, then the 16 MiB bulk copy
        # queued behind them (its descriptors round-robin evenly over all
        # 16 DMA engines, so every engine finishes at the same time).
        # X1: cells 896..1023 on partitions, (example, channel) on free dim.
        # Loaded per example so each transfer reads a fully contiguous
        # 256 KiB run of x (better HBM locality than the strided gather).
        x1 = pool.tile([_NP1, _B * _C], f32, tag="x1")
        for b in range(_B):
            nc.sync.dma_start(
                out=x1[:, b * _C : (b + 1) * _C],
                in_=x_ap[b, _HI:_K, :],
            )
        # X2: cells 0..31.
        x2 = pool.tile([_N, _B * _C], f32, tag="x2")
        nc.sync.dma_start(
            out=x2[:].rearrange("p (b c) -> p b c", b=_B),
            in_=x_ap[:, 0:_N, :].transpose([1, 0, 2]),
        )
        # Bulk copy x -> out[:, :, 0:C].
        nc.sync.dma_start(out=out_ap[:, :, 0:_C], in_=x_ap[:, :, :])

        # Weights on the ACT ring so they land while x1/x2 stream in.
        w = pool.tile([_NP1, 2 * _NNZ], f32, tag="w")
        nc.scalar.dma_start(out=w[:], in_=w_ap)

        y = pool.tile([_NNZ, _B * _C], f32, tag="y")
        for b in range(_B):
            sl = slice(b * _C, (b + 1) * _C)
            ps = psum_pool.tile([_NNZ, _C], f32)
            nc.tensor.matmul(ps[:], w[:, 0:_NNZ], x1[:, sl], start=True, stop=False)
            nc.tensor.matmul(
                ps[:], w[0:_N, _NNZ : 2 * _NNZ], x2[:, sl], start=False, stop=True
            )
            nc.vector.tensor_copy(y[:, sl], ps[:])

        # One store for all of y, dispatched mid-window on the ACT ring.
        nc.scalar.dma_start(
            out=out_ap[:, _Y0:_K, _C : 2 * _C].transpose([1, 0, 2]),
            in_=y[:].rearrange("p (b c) -> p b c", b=_B),
        )

    nc.compile()
    return nc


def _get_nc():
    if "nc" not in _cached:
        _cached["nc"] = _build_nc()
    return _cached["nc"]


def _in_maps(x):
    w1, w2 = _weights()
    w = np.zeros((_NP1, 2 * _NNZ), np.float32)
    w[:, :_NNZ] = w1
    w[:_N, _NNZ:] = w2
    return [
        {"x": np.ascontiguousarray(x[i * _B : (i + 1) * _B]), "w": w}
        for i in range(_NCORES)
    ]


def kernel(x):
    from concourse.bass_utils import run_bass_kernel_spmd

    x = np.asarray(x, dtype=np.float32)
    assert x.shape == (_B_FULL, _K, _C), x.shape
    nc = _get_nc()
    res = run_bass_kernel_spmd(nc, _in_maps(x), list(range(_NCORES)))
    return np.concatenate([r["out"] for r in res.results], axis=0)



# revision 16
# speedup vs baseline: 1.0083x; 1.0083x over previous
"""Trainium2 Bass kernel for NeighborAggregation.

Math: for x of shape (b, k=1024, c=512) viewed as a 32x32 grid over k,
the reference computes y[cell t] = s(t) * 8^(t-1024) where s is a sum of 4
circularly-shifted neighbors minus 4x, and returns concat(x, y) on the c axis.
8^(t-1024) underflows to exactly 0.0 in fp32 for t <= 974, so y is nonzero
only for the last 49 k-rows (t = 975..1023), whose neighbor cells all live in
grid rows {0, 28..31} = flat cells [0..31] and [896..1023].

Kernel strategy (pure data parallel, batch 64 -> 8 cores x 8 examples):
  1. One 16 MiB DRAM->DRAM DMA copies x into out[:, :, 0:512].
  2. The 49 nonzero y rows are computed per example as a sparse fp32 matmul
     on the tensor engine: out49 = W1^T @ x[896:1024] + W2^T @ x[0:32], with
     the neighbor coefficients (+1 x4, -4 self) pre-scaled by 8^(t-1024)
     (exact power-of-two scaling) folded into W. Result lands in
     out[:, 975:1024, 512:1024].
  3. The zero region of y is never written: ExternalOutput buffers are
     pre-zeroed by the runner (both native and PJRT paths).
"""

from contextlib import ExitStack

import numpy as np

_B_FULL, _K, _C = 64, 1024, 512
_NCORES = 8
_B = _B_FULL // _NCORES  # examples per core
_N = 32
_HI = 896  # first cell of grid rows 28..31
_NNZ = 49  # cells 975..1023 have nonzero factor
_Y0 = _K - _NNZ  # 975

_cached = {}
_NP1 = 128  # partitions of the high-cell block (cells 896..1023)


def _weights():
    """W1T (128, 49) over cells 896..1023 and W2T (32, 49) over cells 0..31.

    Column o corresponds to output cell k = 975 + o; entries are the neighbor
    coefficients scaled by factor[k] = 8^(k-1024) (exact in fp32).
    """
    t = np.arange(_K)
    factor = (np.float64(2.0) ** (3.0 * (t - _K))).astype(np.float32)
    w1 = np.zeros((_NP1, _NNZ), np.float32)
    w2 = np.zeros((_N, _NNZ), np.float32)
    for o in range(_NNZ):
        k = _Y0 + o
        i, j = divmod(k, _N)
        f = factor[k]
        i1, i2 = (i + 1) % _N, (i - 2) % _N
        jp, jm = (j + 1) % _N, (j - 2) % _N
        for r, q in [(i1, jp), (i1, jm), (i2, jp), (i2, jm)]:
            cell = _N * r + q
            if cell >= _HI:
                w1[cell - _HI, o] += f
            else:
                w2[cell, o] += f
        w1[k - _HI, o] += np.float32(-4.0) * f
    return w1, w2


def _build_nc():
    import concourse.bacc as bacc
    import concourse.mybir as mybir
    import concourse.tile as tile

    nc = bacc.Bacc("TRN2", debug=False, num_devices=_NCORES)
    f32 = mybir.dt.float32
    x_ap = nc.dram_tensor("x", (_B, _K, _C), f32, kind="ExternalInput").ap()
    w_ap = nc.dram_tensor("w", (_NP1, 2 * _NNZ), f32, kind="ExternalInput").ap()
    out_ap = nc.dram_tensor("out", (_B, _K, 2 * _C), f32, kind="ExternalOutput").ap()

    with tile.TileContext(nc) as tc, ExitStack() as ctx:
        pool = ctx.enter_context(tc.tile_pool(name="sbuf", bufs=1))
        psum_pool = ctx.enter_context(tc.tile_pool(name="psum", bufs=4, space="PSUM"))

        # Sync (SP) ring, in FIFO order: the small matmul inputs FIRST so
        # they run at the full 360 GB/s bus rate, then the 16 MiB bulk copy
        # queued behind them (its descriptors round-robin evenly over all
        # 16 DMA engines, so every engine finishes at the same time).
        # Everything rides the sync (SP) ring: its descriptor deal across the
        # 16 DMA engines is exactly even, unlike the ACT ring whose first
        # ~0.8 MB lands only on engines 64-70 (observed in every trace).
        w = pool.tile([_NP1, 2 * _NNZ], f32, tag="w")
        nc.sync.dma_start(out=w[:], in_=w_ap)

        # X1: cells 896..1023 on partitions, (example, channel) on free dim.
        x1 = pool.tile([_NP1, _B * _C], f32, tag="x1")
        nc.sync.dma_start(
            out=x1[:].rearrange("p (b c) -> p b c", b=_B),
            in_=x_ap[:, _HI:_K, :].transpose([1, 0, 2]),
        )
        # X2: cells 0..31.
        x2 = pool.tile([_N, _B * _C], f32, tag="x2")
        nc.sync.dma_start(
            out=x2[:].rearrange("p (b c) -> p b c", b=_B),
            in_=x_ap[:, 0:_N, :].transpose([1, 0, 2]),
        )
        # Bulk copy x -> out[:, :, 0:C].
        nc.sync.dma_start(out=out_ap[:, :, 0:_C], in_=x_ap[:, :, :])

        y = pool.tile([_NNZ, _B * _C], f32, tag="y")
        for b in range(_B):
            sl = slice(b * _C, (b + 1) * _C)
            ps = psum_pool.tile([_NNZ, _C], f32)
            nc.tensor.matmul(ps[:], w[:, 0:_NNZ], x1[:, sl], start=True, stop=False)
            nc.tensor.matmul(
                ps[:], w[0:_N, _NNZ : 2 * _NNZ], x2[:, sl], start=False, stop=True
            )
            nc.vector.tensor_copy(y[:, sl], ps[:])

        # One store for all of y; its descriptors queue evenly behind the
        # copy's on the sync ring and form the (tiny) tail of the window.
        nc.sync.dma_start(
            out=out_ap[:, _Y0:_K, _C : 2 * _C].transpose([1, 0, 2]),
            in_=y[:].rearrange("p (b c) -> p b c", b=_B),
        )

    nc.compile()
    return nc


def _get_nc():
    if "nc" not in _cached:
        _cached["nc"] = _build_nc()
    return _cached["nc"]


def _in_maps(x):
    w1, w2 = _weights()
    w = np.zeros((_NP1, 2 * _NNZ), np.float32)
    w[:, :_NNZ] = w1
    w[:_N, _NNZ:] = w2
    return [
        {"x": np.ascontiguousarray(x[i * _B : (i + 1) * _B]), "w": w}
        for i in range(_NCORES)
    ]


def kernel(x):
    from concourse.bass_utils import run_bass_kernel_spmd

    x = np.asarray(x, dtype=np.float32)
    assert x.shape == (_B_FULL, _K, _C), x.shape
    nc = _get_nc()
    res = run_bass_kernel_spmd(nc, _in_maps(x), list(range(_NCORES)))
    return np.concatenate([r["out"] for r in res.results], axis=0)

